# revision 64
# baseline (speedup 1.0000x reference)
"""Trainium2 Bass kernel for nn_Attention_29738353557815.

8-way tensor-parallel over heads (core c owns q-heads {2c, 2c+1}, kv-head
c//2), fp16 datapath end to end (PSUM accumulation in f32):

  - hidden^T is DMA'd once as 64 [128, 512] fp16 tiles and stays resident in
    SBUF for all projection passes; all other loads (rope tables, wq blocks)
    are deferred behind it so phase A's k/v chains never wait on DMA.
  - phase order hides each head's AllToAll under later compute:
      [k/v proj] [q0/g0 proj] [h0 attention] -> AllToAll#0
      [q1/g1 proj] [h1 attention]            -> AllToAll#1
      [o-proj ht 0-7 | overlaps A2A#1] [o-proj ht 8-15]
  - k/v projections run dt-major (both accumulators advance per hT tile) so
    the first matmul issues ~1.5us in.
  - attention in S^T layout ([key, query] tiles). Exp runs with bias=-5 so
    pt/rowsum/ot all fit fp16 (max |logit| ~5.2; the shift cancels in the
    normalization). Causal+segment masking is ONE fp16 multiply against a
    host-precomputed {0,1} tile (shared by both heads, resident in SBUF);
    fully-inside tiles skip masking, fully-outside tiles are skipped at
    build time.
  - st matmuls are emitted 3 tiles ahead of their PV/rowsum consumers so the
    PE never drains while ACT/DVE process the softmax tiles.
  - normalization/gating: gpT = 1+exp(-gate) is precomputed per head;
    the chunk epilogue computes atg = ot * 1/(gpT * rowsum) with one
    fast-reciprocal — no Ln/Exp on the ACT engine inside attention, keeping
    ACT strictly below the PE rate there.
  - o-proj accumulates ht-major across 8 PSUM banks; the second half runs
    Dc-major so output assembly/DMA overlaps the tail accumulation. The
    ht=0 matmuls carry a scheduling-pass floor (tile_wait_until) so the
    scheduler cannot interleave collective-gated o-proj matmuls ahead of
    h1's attention in the in-order PE stream.
"""
import sys

if "/opt/trn_rl_repo" not in sys.path:
    sys.path.insert(0, "/opt/trn_rl_repo")

import numpy as np

import concourse.bass as bass
import concourse.bass_isa as bass_isa
from concourse import bacc
import concourse.mybir as mybir
import concourse.tile as tile
from concourse.bass_utils import run_bass_kernel_spmd
from concourse.masks import make_identity

F32 = mybir.dt.float32
F16 = mybir.dt.float16
AF = mybir.ActivationFunctionType
OP = mybir.AluOpType

B, T, D = 1, 2048, 2048
NH, NKV, HD = 16, 4, 128
EPS = 1e-6
SCALE = HD ** -0.5
NCORES = 8
P = 128
NJ = T // 512      # 4 t-chunks of 512
NT = T // P        # 16 s-tiles of 128
DT = D // P        # 16 contraction tiles
TSL = T // NCORES  # 256 output rows per core
EXP_BIAS = -5.0    # keeps exp(logit) in fp16 range; cancels in softmax
OPROJ_FLOOR_MS = 0.135  # scheduling floor for o-proj ht0 matmuls

_program_cache: dict = {}


def _tile_flags(seg_end: np.ndarray):
    """Per (s-tile i, t-chunk j): (skip, needs_causal, needs_seg, c0, c1).

    [c0, c1) is the live query-column window: columns below c0 are dead by
    causality (t < every s in the tile), columns at/above c1 are dead by
    segment end. Dead columns are sliced out of st/exp/mask/PV work; the
    first valid tile of each chunk always runs full width to initialize the
    PSUM accumulators (its c0 is provably 0)."""
    flags = []
    for i in range(NT):
        smin, smax = P * i, P * i + P - 1
        se_lo = int(seg_end[smin])
        se_hi = int(seg_end[smax])
        row = []
        for j in range(NJ):
            t0, t1 = 512 * j, 512 * j + 511
            skip = (t1 < smin) or (t0 >= se_hi)
            causal = (not skip) and (t0 < smax)
            segm = (not skip) and (t1 >= se_lo)
            c0 = max(0, min(512, smin - t0))
            c1 = max(0, min(512, se_hi - t0))
            row.append((skip, causal, segm, c0, c1))
        flags.append(row)
    return tuple(tuple(r) for r in flags)


def _masked_tiles(flags):
    """[(i, j, base, width)] needing a mask multiply, in slot order.

    Stored window is [base, base+width): full 512 when the tile can be a
    chunk's first valid tile (those run full-width to init PSUM), else the
    live [c0, c1) window."""
    out = []
    for j in range(NJ):
        vs = [i for i in range(NT) if not flags[i][j][0]]
        for i in vs:
            _, nc_, ns_, c0, c1 = flags[i][j]
            if not (nc_ or ns_):
                continue
            if i == vs[0]:
                out.append((i, j, 0, 512))
            else:
                out.append((i, j, c0, c1 - c0))
    return out


def _build_program(key, use_collective=True):
    flags, unit_w = key
    mtiles = _masked_tiles(flags)
    mslot = {(i, j): (s, b, w) for s, (i, j, b, w) in enumerate(mtiles)}
    mtot = sum(w for _, _, _, w in mtiles)
    nc = bacc.Bacc("TRN2", target_bir_lowering=False, debug=False,
                   num_devices=NCORES)

    hT_d = nc.dram_tensor("hT", [P, DT, T], F16, kind="ExternalInput")
    # host-prepacked, column-block-major: [q0 | q1 | g0 | g1] each [P, DT, 128]
    wqg_d = nc.dram_tensor("wqg", [2, P, DT, P], F16, kind="ExternalInput")
    wqg8_d = nc.dram_tensor("wqg8", [NT, P, DT, P], F16, kind="ExternalInput")
    hts_d = nc.dram_tensor("hts", [P, DT, TSL], F16, kind="ExternalInput")
    wkv_d = nc.dram_tensor("wkv", [2, P, DT, P], F16, kind="ExternalInput")
    wo_d = nc.dram_tensor("wo", [P, NT, 2048], F16, kind="ExternalInput")
    tblq_d = nc.dram_tensor("tblq", [2, P, T], F16, kind="ExternalInput")
    if not unit_w:
        wqk_d = nc.dram_tensor("wqk", [P, 2], F32, kind="ExternalInput")
    masks_d = nc.dram_tensor("masks", [P, max(1, mtot)], F16,
                             kind="ExternalInput")
    out_d = nc.dram_tensor("out", [TSL, D], F32, kind="ExternalOutput")

    with tile.TileContext(nc) as tc:
        with (
            tc.tile_pool(name="consts", bufs=1) as consts,
            tc.tile_pool(name="perm", bufs=1) as perm,
            tc.tile_pool(name="hw", bufs=4) as hw,
            tc.tile_pool(name="tmp", bufs=3) as tmp,
            tc.tile_pool(name="ptp", bufs=4) as ptp,
            tc.tile_pool(name="ps", bufs=1, space="PSUM") as psp,
            tc.tile_pool(name="dram", bufs=1, space="DRAM") as dram,
        ):
            # ---- resident hidden^T tiles: hTt[dt][j] = hT[:, dt, 512j:+512]
            hTt = [[consts.tile([P, 512], F16, tag=f"hT_{dt}_{j}",
                                name=f"hT_{dt}_{j}") for j in range(NJ)]
                   for dt in range(DT)]
            wq_sb = [consts.tile([P, DT, P], F16, tag=f"wq{c}", name=f"wq{c}")
                     for c in range(2)]        # q0, q1 column blocks
            hts_sb = consts.tile([P, DT, TSL], F16, tag="hts", name="hts")
            wkv_sb = [consts.tile([P, DT, P], F16, tag=f"wkv{c}",
                                  name=f"wkv{c}") for c in range(2)]  # k, v
            tb = {}
            for nm, idx in (("c", 0), ("s", 1)):
                tb[nm] = consts.tile([P, T], F16, tag=f"tb_{nm}",
                                     name=f"tb_{nm}")
            if not unit_w:
                wqk_sb = consts.tile([P, 2], F32)
                nc.sync.dma_start(wqk_sb[:], wqk_d[:])
            mask_sb = [consts.tile([P, w], F16, tag=f"mask{s}",
                                    name=f"mask{s}")
                       for s, (_, _, _, w) in enumerate(mtiles)]
            ones_sb = consts.tile([P, P], F16)
            nc.vector.memset(ones_sb[:], 1.0)
            ident_sb = consts.tile([P, P], F16)
            make_identity(nc, ident_sb[:])
            eps_sb = consts.tile([P, 1], F32)
            nc.vector.memset(eps_sb[:], EPS)
            eb_sb = consts.tile([P, 1], F32)
            nc.vector.memset(eb_sb[:], EXP_BIAS)
            zcol_sb = consts.tile([P, 1], F16)
            nc.vector.memset(zcol_sb[:], 0.0)

            # ---- persistent activations ----
            qTr = [perm.tile([P, T], F16, tag=f"qTr{h}", name=f"qTr{h}")
                   for h in range(2)]
            kTr = perm.tile([P, T], F16, tag="kTr")
            v_sb = perm.tile([P, NT, P], F16, tag="v_sb")
            rcg8 = perm.tile([P, 8, TSL], F16, tag="rcg8")  # h1 gate recips

            a2a_in = [dram.tile([NCORES * P, TSL], F16, name=f"a2a_in{h}")
                      for h in range(2)]
            a2a_in8 = [a.rearrange("(s r) t -> s r t", r=P) for a in a2a_in]
            a2a_out = [dram.tile([NCORES * P, TSL], F16, name=f"a2a_out{h}")
                       for h in range(2)]

            # ================= DMA schedule (SP FIFO order) =================
            # wkv quartiles pace the first k/v chains; all of hT streams next;
            # everything else (wq, rope tables, masks) is needed only after
            # phase A finishes (~30us) and loads behind it.
            def dma_hT(j, dts):
                for dt in dts:
                    nc.sync.dma_start(hTt[dt][j][:],
                                      hT_d[:, dt, 512 * j:512 * j + 512])

            nc.sync.dma_start(wkv_sb[0][:], wkv_d[0])
            dma_hT(0, range(DT))
            nc.sync.dma_start(wkv_sb[1][:], wkv_d[1])
            dma_hT(1, range(DT))
            for nm, idx in (("c", 0), ("s", 1)):
                nc.sync.dma_start(tb[nm][:], tblq_d[idx])
            dma_hT(2, range(DT))
            nc.sync.dma_start(wq_sb[0][:], wqg_d[0])
            dma_hT(3, range(DT))
            nc.sync.dma_start(wq_sb[1][:], wqg_d[1])
            nc.sync.dma_start(hts_sb[:], hts_d[:])
            moff = 0
            for s, (_, _, _, w) in enumerate(mtiles):
                nc.sync.dma_start(mask_sb[s][:], masks_d[:, moff:moff + w])
                moff += w
            # first wg-ring slots prefetch here; the rest stream in phase D
            wg_sb = []
            for bn in range(4):
                w_ = hw.tile([P, DT, P], F16, tag="wg", bufs=3,
                             name=f"wg_{bn}")
                nc.sync.dma_start(w_[:], wqg8_d[bn])
                wg_sb.append(w_)

            # ================= shared epilogues =================
            def rope_norm_epi(mm_ps, dest, tsl, widx):
                """rms-norm + rope from a [P,512] PSUM proj block to dest.

                qpre gets a private ring: it is the PSUM drain, and must
                never wait behind rope ops stalled on the table DMAs."""
                qpre = tmp.tile([P, 512], F16, tag="qpre", bufs=3)
                nc.vector.tensor_copy(qpre[:], mm_ps[:])
                q2 = tmp.tile([P, 512], F16, tag="tmp2", bufs=2)
                nc.scalar.activation(q2[:], mm_ps[:], AF.Square)
                if not unit_w:
                    qw = tmp.tile([P, 512], F16, tag="tmp")
                    nc.vector.tensor_scalar_mul(
                        qw[:], qpre[:], wqk_sb[:, widx:widx + 1])
                    qpre = qw
                ssq_ps = psp.tile([P, 512], F32, tag="aux", bufs=1)
                nc.tensor.matmul(ssq_ps[:], ones_sb[:], q2[:],
                                 start=True, stop=True)
                rsv = tmp.tile([P, 512], F16, tag="tmp")
                nc.scalar.activation(rsv[:], ssq_ps[:], AF.Ln,
                                     scale=1.0 / HD, bias=eps_sb[:, 0:1])
                nc.scalar.activation(rsv[:], rsv[:], AF.Exp, scale=-0.5)
                tcos = tmp.tile([P, 512], F16, tag="tmp")
                nc.vector.tensor_tensor(tcos[:], qpre[:], tb["c"][:, tsl],
                                        OP.mult)
                t2 = tmp.tile([P, 512], F16, tag="tmp")
                # sin halves pre-swapped host-side; only out is shifted
                nc.vector.tensor_tensor(t2[0:64, :], qpre[64:128, :],
                                        tb["s"][64:128, tsl], OP.mult)
                nc.vector.tensor_tensor(t2[64:128, :], qpre[0:64, :],
                                        tb["s"][0:64, tsl], OP.mult)
                nc.vector.tensor_tensor(t2[:], tcos[:], t2[:], OP.add)
                nc.vector.tensor_tensor(dest, t2[:], rsv[:], OP.mult)

            def v_epi(mm_ps, j):
                # private ring: vtmp drains PSUM and feeds PE transposes; it
                # must not queue behind rope ops stalled on table DMAs
                vtmp = tmp.tile([P, 512], F16, tag="vtmp", bufs=2)
                nc.vector.tensor_copy(vtmp[:], mm_ps[:])
                for kk in range(4):
                    tt = 4 * j + kk
                    trp = psp.tile([P, P], F16, tag="aux", bufs=1)
                    nc.tensor.transpose(
                        trp[:], vtmp[:, P * kk:P * kk + P], ident_sb[:])
                    nc.vector.tensor_copy(v_sb[:, tt, :], trp[:])

            # ================= phase A: k/v projections =================
            # j0 runs k-chain then v-chain (k needs only the first weight
            # DMA, so the PE starts ~1.5us in and the v weights hide behind
            # j0's tile stream); later chunks interleave k/v per tile.
            for j in range(NJ):
                tsl = slice(512 * j, 512 * j + 512)
                kps = psp.tile([P, 512], F32, tag="mm", bufs=3,
                               name=f"kps{j}")
                vps = psp.tile([P, 512], F32, tag="mm", bufs=3,
                               name=f"vps{j}")
                if j == 0:
                    for dt in range(DT):
                        nc.tensor.matmul(kps[:], wkv_sb[0][:, dt, :],
                                         hTt[dt][j][:],
                                         start=(dt == 0), stop=(dt == DT - 1))
                    for dt in range(DT):
                        nc.tensor.matmul(vps[:], wkv_sb[1][:, dt, :],
                                         hTt[dt][j][:],
                                         start=(dt == 0), stop=(dt == DT - 1))
                else:
                    for dt in range(DT):
                        nc.tensor.matmul(kps[:], wkv_sb[0][:, dt, :],
                                         hTt[dt][j][:],
                                         start=(dt == 0), stop=(dt == DT - 1))
                        nc.tensor.matmul(vps[:], wkv_sb[1][:, dt, :],
                                         hTt[dt][j][:],
                                         start=(dt == 0), stop=(dt == DT - 1))
                rope_norm_epi(kps, kTr[:, tsl], tsl, widx=1)
                v_epi(vps, j)

            # ================= phases per head =================
            def proj_qg(h):
                for j in range(NJ):
                    tsl = slice(512 * j, 512 * j + 512)
                    qps = psp.tile([P, 512], F32, tag="mm", bufs=3,
                                   name=f"qps{h}{j}")
                    for dt in range(DT):
                        nc.tensor.matmul(qps[:], wq_sb[h][:, dt, :],
                                         hTt[dt][j][:],
                                         start=(dt == 0), stop=(dt == DT - 1))
                    rope_norm_epi(qps, qTr[h][:, tsl], tsl, widx=0)

            def emit_attention(h, j):
                tsl = slice(512 * j, 512 * j + 512)
                valid = [i for i in range(NT) if not flags[i][j][0]]
                last = len(valid) - 1
                ot_ps = psp.tile([P, 512], F32, tag="acc", bufs=4,
                                 name=f"ot_{h}_{j}")
                # softmax denominator accumulates on the Pool engine where
                # that engine is free: all of h0 (before AllToAll#0), and
                # h1's last chunks (whose pt tiles appear only after A2A#0
                # has released Pool). h1's early chunks keep PE ones-matmuls:
                # their Pool ops would queue behind the 28us collective and
                # back-pressure the pt ring into a PE stall.
                pool_rs = (h == 0)
                if pool_rs:
                    rsacc = tmp.tile([P, 512], F16, tag="rsacc", bufs=2,
                                     name=f"rsacc_{h}_{j}")
                else:
                    rs_ps = psp.tile([P, 512], F32, tag="acc", bufs=4,
                                     name=f"rs_{h}_{j}")
                DEPTH = 3  # st matmuls emitted ahead of their PV consumers
                pts = {}

                def win(idx):
                    # live column window; first tile full-width (PSUM init)
                    if idx == 0:
                        return slice(0, 512)
                    _, _, _, c0, c1 = flags[valid[idx]][j]
                    return slice(c0, c1)

                def front(idx):
                    i = valid[idx]
                    w = win(idx)
                    st_ps = psp.tile([P, 512], F32, tag="mm", bufs=3,
                                     name=f"st_{h}_{j}_{i}")
                    nc.tensor.matmul(st_ps[:, w], kTr[:, P * i:P * i + P],
                                     qTr[h][:, 512 * j + w.start:
                                            512 * j + w.stop],
                                     start=True, stop=True)
                    pt = ptp.tile([P, 512], F16, tag="pt",
                                  name=f"pt_{h}_{j}_{i}")
                    nc.scalar.activation(pt[:, w], st_ps[:, w], AF.Exp,
                                         bias=eb_sb[:, 0:1])
                    if (i, j) in mslot:
                        ms, mb, _ = mslot[(i, j)]
                        nc.vector.tensor_tensor(
                            pt[:, w], pt[:, w],
                            mask_sb[ms][:, w.start - mb:w.stop - mb],
                            OP.mult)
                    pts[idx] = pt

                def back(idx):
                    i = valid[idx]
                    w = win(idx)
                    pt = pts.pop(idx)
                    nc.tensor.matmul(ot_ps[:, w], v_sb[:, i, :], pt[:, w],
                                     start=(idx == 0), stop=(idx == last))
                    if pool_rs:
                        if idx == 0:
                            nc.gpsimd.tensor_copy(rsacc[:], pt[:])
                        else:
                            nc.gpsimd.tensor_tensor(rsacc[:, w], rsacc[:, w],
                                                    pt[:, w], OP.add)
                    else:
                        nc.tensor.matmul(rs_ps[:, w], ones_sb[:], pt[:, w],
                                         start=(idx == 0), stop=(idx == last))

                for idx in range(len(valid)):
                    front(idx)
                    if idx >= DEPTH - 1:
                        back(idx - DEPTH + 1)
                for idx in range(max(0, len(valid) - DEPTH + 1), len(valid)):
                    back(idx)
                if pool_rs:
                    rsall = tmp.tile([P, 512], F32, tag="rsall", bufs=1,
                                     name=f"rsall_{h}_{j}")
                    nc.gpsimd.partition_all_reduce(
                        rsall[:], rsacc[:], channels=P,
                        reduce_op=bass_isa.ReduceOp.add)
                    rs_ap = rsall[:]
                else:
                    rs_ap = rs_ps[:]

                # atg = ot / rowsum (gating applied at the destination);
                # per 256-col half so each shard's staging DMA launches as
                # soon as its half of the epilogue is done
                sgm = tmp.tile([P, 512], F32, tag="tmp", name=f"sgm_{h}_{j}")
                atg = tmp.tile([P, 512], F16, tag="tmp2", bufs=2,
                               name=f"atg_{h}_{j}")
                for half in range(2):
                    hs = slice(256 * half, 256 * half + 256)
                    nc.vector.reciprocal_approx_fast(sgm[:, hs], rs_ap[:, hs])
                    nc.vector.tensor_tensor(atg[:, hs], ot_ps[:, hs],
                                            sgm[:, hs], OP.mult)
                    nc.sync.dma_start(
                        a2a_in8[h][2 * j + half, :, :], atg[:, hs])
                return atg

            def a2a(h):
                if use_collective:
                    nc.gpsimd.collective_compute(
                        "AllToAll", OP.bypass,
                        replica_groups=[list(range(NCORES))],
                        ins=[a2a_in[h][:].opt()], outs=[a2a_out[h][:].opt()])
                else:
                    nc.sync.dma_start(a2a_out[h][:], a2a_in[h][:])

            proj_qg(0)
            for j in range(NJ):
                emit_attention(0, j)
            a2a(0)
            # wo prefetch, first half only: in the SP queue after h0's
            # staging and BEFORE h1's (the second half is emitted in phase D
            # so h1's staging DMAs are never queued behind it)
            wo_sb = []
            for ht in range(8):
                w_ = hw.tile([P, 2048], F16, tag="wo", bufs=8, name=f"wo_{ht}")
                nc.sync.dma_start(w_[:], wo_d[:, ht, :])
                wo_sb.append(w_)
            proj_qg(1)
            last_atg = None
            for j in range(NJ):
                last_atg = emit_attention(1, j)
            a2a(1)

            # ================= o-proj =================
            # 8 PSUM banks accumulate [m 0/1] x [Dc 0..3] over all 16 ht
            # blocks; part 1 (ht 0-7, from a2a_out[0]) runs while A2A#1 is in
            # flight; part 2 runs Dc-major so assembly overlaps the tail.
            ops_tags = ["mm", "mm", "mm", "aux", "acc", "acc", "acc", "acc"]
            ops_bufs = {"mm": 3, "aux": 1, "acc": 4}
            ops = []
            for m in range(2):
                for Dc in range(NJ):
                    tg = ops_tags[m * NJ + Dc]
                    ops.append(psp.tile([P, 512], F32, tag=tg,
                                        bufs=ops_bufs[tg],
                                        name=f"ops{m}_{Dc}"))
            at0 = perm.tile([P, 8, TSL], F16, tag="ATall0", name="ATall0")
            nc.sync.dma_start(
                at0[:], a2a_out[0].rearrange("(i r) t -> r i t", r=P))
            # order the gating (and transitively all o-proj matmuls) behind
            # h1's attention with a REAL data dep: a 1.0 derived from h1's
            # last atg tile scales at0's first block in place
            one1 = tmp.tile([P, 1], F32, tag="rcg0", bufs=2, name="one1")
            nc.vector.tensor_scalar(one1[:], last_atg[:, 0:1], 0.0, 1.0,
                                    OP.mult, OP.add)
            nc.vector.tensor_scalar_mul(at0[:, 0, :], at0[:, 0, :], one1[:])

            # ---- destination-side gating: per received head block ht,
            # project THIS core's 256 hidden rows through that head's gate
            # columns (16 matmuls of [P,256]), then scale the block by
            # sigmoid = 1/(1+e^-gate) in place. All projections run during
            # the A2A windows; h0 blocks gate as soon as rcpg is ready,
            # h1 blocks gate when at1 lands.
            rcpgs = []
            for bn in range(NT):
                if bn >= 4:
                    w_ = hw.tile([P, DT, P], F16, tag="wg", bufs=3,
                                 name=f"wg_{bn}")
                    nc.sync.dma_start(w_[:], wqg8_d[bn])
                    wg_sb.append(w_)
                gate_ps = psp.tile([P, TSL], F32, tag="mm", bufs=3,
                                   name=f"gps_{bn}")
                for dt in range(DT):
                    nc.tensor.matmul(gate_ps[:], wg_sb[bn][:, dt, :],
                                     hts_sb[:, dt, :],
                                     start=(dt == 0), stop=(dt == DT - 1))
                eg = tmp.tile([P, TSL], F16, tag="tmp", name=f"eg_{bn}")
                nc.scalar.activation(eg[:], gate_ps[:], AF.Exp, scale=-1.0)
                gp1 = tmp.tile([P, TSL], F32, tag="tmp", name=f"gp1_{bn}")
                nc.vector.tensor_scalar_add(gp1[:], eg[:], 1.0)
                if bn < 8:
                    rcpg = tmp.tile([P, TSL], F32, tag="rcg0", bufs=2,
                                    name=f"rcpg_{bn}")
                    nc.vector.reciprocal_approx_fast(rcpg[:], gp1[:])
                    nc.vector.tensor_tensor(at0[:, bn, :], at0[:, bn, :],
                                            rcpg[:], OP.mult)
                else:
                    rcpg = tmp.tile([P, TSL], F32, tag="rcg0", bufs=2,
                                    name=f"rcpg_{bn}")
                    nc.vector.reciprocal_approx_fast(rcpg[:], gp1[:])
                    nc.vector.tensor_copy(rcg8[:, bn - 8, :], rcpg[:])
            # second wo half loads here: after h1's staging in the SP queue,
            # paced by o-proj part 1 freeing ring slots
            for ht in range(8, NT):
                w_ = hw.tile([P, 2048], F16, tag="wo", bufs=8, name=f"wo_{ht}")
                nc.sync.dma_start(w_[:], wo_d[:, ht, :])
                wo_sb.append(w_)
            at1 = perm.tile([P, 8, TSL], F16, tag="ATall1", name="ATall1")
            nc.sync.dma_start(
                at1[:], a2a_out[1].rearrange("(i r) t -> r i t", r=P))
            for bn in range(8, NT):
                nc.vector.tensor_tensor(at1[:, bn - 8, :], at1[:, bn - 8, :],
                                        rcg8[:, bn - 8, :], OP.mult)
            ATall = [at0, at1]

            def oproj_mm(ht, Dc, m):
                nc.tensor.matmul(
                    ops[m * NJ + Dc][:],
                    ATall[ht // 8][:, ht % 8, P * m:P * m + P],
                    wo_sb[ht][:, 512 * Dc:512 * Dc + 512],
                    start=(ht == 0), stop=(ht == NT - 1))

            for ht in range(8):
                for Dc in range(NJ):
                    for m in range(2):
                        oproj_mm(ht, Dc, m)
            # keep the PE p-state warm across the A2A#1 wait: a stream of
            # zero-accumulate matmuls (lhsT = a zero column, +0 into row 0 of
            # an open bank) fills the idle window so part 2 runs at full rate
            for w in range(100):
                nc.tensor.matmul(ops[0][0:1, :], zcol_sb[:],
                                 hTt[0][0][:], start=False, stop=False)
            # part 2: Dc-major so each closed [m, Dc] block drains and
            # streams out while later Dc blocks still accumulate
            for Dc in range(NJ):
                for m in range(2):
                    for ht in range(8, NT):
                        oproj_mm(ht, Dc, m)
                for m in range(2):
                    o_sb = hw.tile([P, 512], F32, tag="osb", bufs=2,
                                   name=f"o_{m}_{Dc}")
                    nc.vector.tensor_copy(o_sb[:], ops[m * NJ + Dc][:])
                    nc.sync.dma_start(
                        out_d[P * m:P * m + P, 512 * Dc:512 * Dc + 512],
                        o_sb[:])

    nc.compile()
    _dedupe_act_table_loads(nc)
    return nc


def _dedupe_act_table_loads(nc):
    """Bacc assigns Exp->exp_and_others and Ln->natural_log, inserting a
    ~2.7us table load at every Exp<->Ln alternation. All activation funcs
    this kernel uses (Exp, Ln, Square) live in the natural_log_exp_and_others
    set, so keep one load of that set and drop the rest."""
    from concourse.hw_specs import get_activation_tables
    tabs = list(get_activation_tables(nc.m.arch).items())
    nl_exp = next(i for i, (nm, funcs) in enumerate(tabs)
                  if nm == "natural_log_exp_and_others")
    used = {ins.func for bb in nc.main_func.blocks for ins in bb.instructions
            if isinstance(ins, mybir.InstActivation)}
    assert used <= tabs[nl_exp][1], f"funcs {used} not all in natural_log_exp"
    first = True
    for bb in nc.main_func.blocks:
        keep = []
        for ins in bb.instructions:
            if isinstance(ins, mybir.InstLoadActFuncSet):
                assert ins.sync_info is None or (
                    not ins.sync_info.on_wait and not ins.sync_info.on_update)
                if first:
                    ins.act_func_set_id = nl_exp
                    keep.append(ins)
                    first = False
                continue
            keep.append(ins)
        bb.instructions[:] = keep


def _host_prep(hidden_BTD, cos_BTK, sin_BTK, segment_ids_BT, position_ids_BT,
               wq, wk, wv, wo, q_norm_w, k_norm_w):
    hidden = np.ascontiguousarray(np.asarray(hidden_BTD, dtype=np.float32)[0])
    cos = np.asarray(cos_BTK, dtype=np.float32)[0]
    sin = np.asarray(sin_BTK, dtype=np.float32)[0]
    seg = np.asarray(segment_ids_BT)[0]
    pos = np.asarray(position_ids_BT)[0]
    wq = np.asarray(wq, dtype=np.float32)
    wk = np.asarray(wk, dtype=np.float32)
    wv = np.asarray(wv, dtype=np.float32)
    wo = np.asarray(wo, dtype=np.float32)
    q_norm_w = np.asarray(q_norm_w, dtype=np.float32)
    k_norm_w = np.asarray(k_norm_w, dtype=np.float32)

    assert np.array_equal(pos, np.arange(T, dtype=pos.dtype)), \
        "kernel assumes position_ids == arange"
    assert np.all(np.diff(seg) >= 0), "kernel assumes sorted segment ids"

    # hT[p, dt, t] = hidden[t, 128*dt + p]
    hT = np.ascontiguousarray(
        hidden.T.reshape(DT, P, T).transpose(1, 0, 2).astype(np.float16))
    sqrtS = np.float32(np.sqrt(SCALE))
    signv = np.where(np.arange(HD) < HD // 2, -1.0, 1.0).astype(np.float32)
    shuf = (np.arange(HD) + HD // 2) % HD

    cosw = (cos.T * sqrtS).astype(np.float32)
    sinw = (sin.T * signv[:, None] * sqrtS).astype(np.float32)
    sinswap = sinw[shuf]  # halves swapped: see rope ops in _build_program
    tblq = np.ascontiguousarray(np.stack([cosw, sinswap]).astype(np.float16))
    unit_w = bool(np.all(q_norm_w == 1.0) and np.all(k_norm_w == 1.0))
    wqk = np.ascontiguousarray(np.stack([q_norm_w, k_norm_w], axis=1))

    # prepack wo: partition-major, block order = o-proj ht order
    # (all h0 head-blocks, then all h1)
    perm = [2 * i + h for h in range(2) for i in range(NCORES)]
    wo_p = wo.reshape(NT, P, 2048)[perm].transpose(1, 0, 2)
    wo_p = np.ascontiguousarray(wo_p.astype(np.float16))

    seg_end = np.searchsorted(seg, seg, side="right").astype(np.int64)
    flags = _tile_flags(seg_end)
    mtiles = _masked_tiles(flags)
    # packed mask windows: keep iff 128i+p <= 512j+t < seg_end[128i+p]
    mtot = sum(w for _, _, _, w in mtiles)
    masks = np.zeros((P, max(1, mtot)), dtype=np.float16)
    moff = 0
    for (i, j, b, w) in mtiles:
        sglob = P * i + np.arange(P)
        tg = 512 * j + b + np.arange(w)
        keep = (sglob[:, None] <= tg[None, :]) & \
               (tg[None, :] < seg_end[sglob][:, None])
        masks[:, moff:moff + w] = keep.astype(np.float16)
        moff += w

    # gate columns for ALL heads, in o-proj block order (shared by cores)
    gperm = [2 * i + hh for hh in range(2) for i in range(NCORES)]
    wqg8 = np.stack([
        wq[:, ht * 256 + 128: ht * 256 + 256].reshape(DT, P, P)
        .transpose(1, 0, 2) for ht in gperm])
    wqg8 = np.ascontiguousarray(wqg8.astype(np.float16))

    in_maps = []
    for c in range(NCORES):
        h0, h1 = 2 * c, 2 * c + 1
        g = c // 2
        cols = [wq[:, h0 * 256: h0 * 256 + 128],
                wq[:, h1 * 256: h1 * 256 + 128]]
        wqg_p = np.stack([cb.reshape(DT, P, P).transpose(1, 0, 2)
                          for cb in cols])
        wqg_p = np.ascontiguousarray(wqg_p.astype(np.float16))
        kvcols = [wk[:, g * 128:(g + 1) * 128], wv[:, g * 128:(g + 1) * 128]]
        wkv_p = np.stack([cb.reshape(DT, P, P).transpose(1, 0, 2)
                          for cb in kvcols])
        wkv_p = np.ascontiguousarray(wkv_p.astype(np.float16))
        # this core's 256 hidden^T columns, for destination-side gating
        hts = np.ascontiguousarray(hT[:, :, TSL * c:TSL * (c + 1)])
        m = {
            "hT": hT, "wqg": wqg_p, "wkv": wkv_p, "wo": wo_p,
            "tblq": tblq, "masks": masks, "wqg8": wqg8, "hts": hts,
        }
        if not unit_w:
            m["wqk"] = wqk
        in_maps.append(m)
    return in_maps, seg_end, unit_w


def kernel(**inputs) -> np.ndarray:
    in_maps, seg_end, unit_w = _host_prep(**inputs)
    key = (_tile_flags(seg_end), unit_w)
    if key not in _program_cache:
        _program_cache[key] = _build_program(key)
    nc = _program_cache[key]
    res = run_bass_kernel_spmd(nc, in_maps, list(range(NCORES)))
    out = np.concatenate([res.results[c]["out"] for c in range(NCORES)], axis=0)
    return out[None].astype(np.float32)


# revision 66
# speedup vs baseline: 1.1117x; 1.1117x over previous
"""Trainium2 Bass kernel for nn_Attention_29738353557815.

8-way tensor-parallel over heads (core c owns q-heads {2c, 2c+1}, kv-head
c//2), fp16 datapath end to end (PSUM accumulation in f32):

  - hidden^T is DMA'd once as 64 [128, 512] fp16 tiles and stays resident in
    SBUF for all projection passes; all other loads (rope tables, wq blocks)
    are deferred behind it so phase A's k/v chains never wait on DMA.
  - phase order hides each head's AllToAll under later compute:
      [k/v proj] [q0/g0 proj] [h0 attention] -> AllToAll#0
      [q1/g1 proj] [h1 attention]            -> AllToAll#1
      [o-proj ht 0-7 | overlaps A2A#1] [o-proj ht 8-15]
  - k/v projections run dt-major (both accumulators advance per hT tile) so
    the first matmul issues ~1.5us in.
  - attention in S^T layout ([key, query] tiles). Exp runs with bias=-5 so
    pt/rowsum/ot all fit fp16 (max |logit| ~5.2; the shift cancels in the
    normalization). Causal+segment masking is ONE fp16 multiply against a
    host-precomputed {0,1} tile (shared by both heads, resident in SBUF);
    fully-inside tiles skip masking, fully-outside tiles are skipped at
    build time.
  - st matmuls are emitted 3 tiles ahead of their PV/rowsum consumers so the
    PE never drains while ACT/DVE process the softmax tiles.
  - normalization/gating: gpT = 1+exp(-gate) is precomputed per head;
    the chunk epilogue computes atg = ot * 1/(gpT * rowsum) with one
    fast-reciprocal — no Ln/Exp on the ACT engine inside attention, keeping
    ACT strictly below the PE rate there.
  - o-proj accumulates ht-major across 8 PSUM banks; the second half runs
    Dc-major so output assembly/DMA overlaps the tail accumulation. The
    ht=0 matmuls carry a scheduling-pass floor (tile_wait_until) so the
    scheduler cannot interleave collective-gated o-proj matmuls ahead of
    h1's attention in the in-order PE stream.
"""
import sys

if "/opt/trn_rl_repo" not in sys.path:
    sys.path.insert(0, "/opt/trn_rl_repo")

import numpy as np

import concourse.bass as bass
import concourse.bass_isa as bass_isa
from concourse import bacc
import concourse.mybir as mybir
import concourse.tile as tile
from concourse.bass_utils import run_bass_kernel_spmd
from concourse.masks import make_identity

F32 = mybir.dt.float32
F16 = mybir.dt.float16
AF = mybir.ActivationFunctionType
OP = mybir.AluOpType

B, T, D = 1, 2048, 2048
NH, NKV, HD = 16, 4, 128
EPS = 1e-6
SCALE = HD ** -0.5
NCORES = 8
P = 128
NJ = T // 512      # 4 t-chunks of 512
NT = T // P        # 16 s-tiles of 128
DT = D // P        # 16 contraction tiles
TSL = T // NCORES  # 256 output rows per core
EXP_BIAS = -5.0    # keeps exp(logit) in fp16 range; cancels in softmax
OPROJ_FLOOR_MS = 0.135  # scheduling floor for o-proj ht0 matmuls

_program_cache: dict = {}


def _tile_flags(seg_end: np.ndarray):
    """Per (s-tile i, t-chunk j): (skip, needs_causal, needs_seg, c0, c1).

    [c0, c1) is the live query-column window: columns below c0 are dead by
    causality (t < every s in the tile), columns at/above c1 are dead by
    segment end. Dead columns are sliced out of st/exp/mask/PV work; the
    first valid tile of each chunk always runs full width to initialize the
    PSUM accumulators (its c0 is provably 0)."""
    flags = []
    for i in range(NT):
        smin, smax = P * i, P * i + P - 1
        se_lo = int(seg_end[smin])
        se_hi = int(seg_end[smax])
        row = []
        for j in range(NJ):
            t0, t1 = 512 * j, 512 * j + 511
            skip = (t1 < smin) or (t0 >= se_hi)
            causal = (not skip) and (t0 < smax)
            segm = (not skip) and (t1 >= se_lo)
            c0 = max(0, min(512, smin - t0))
            c1 = max(0, min(512, se_hi - t0))
            row.append((skip, causal, segm, c0, c1))
        flags.append(row)
    return tuple(tuple(r) for r in flags)


def _masked_tiles(flags):
    """[(i, j, base, width)] needing a mask multiply, in slot order.

    Stored window is [base, base+width): full 512 when the tile can be a
    chunk's first valid tile (those run full-width to init PSUM), else the
    live [c0, c1) window."""
    out = []
    for j in range(NJ):
        vs = [i for i in range(NT) if not flags[i][j][0]]
        for i in vs:
            _, nc_, ns_, c0, c1 = flags[i][j]
            if not (nc_ or ns_):
                continue
            if i == vs[0]:
                out.append((i, j, 0, 512))
            else:
                out.append((i, j, c0, c1 - c0))
    return out


def _build_program(key, use_collective=True):
    flags, unit_w = key
    mtiles = _masked_tiles(flags)
    mslot = {(i, j): (s, b, w) for s, (i, j, b, w) in enumerate(mtiles)}
    mtot = sum(w for _, _, _, w in mtiles)
    nc = bacc.Bacc("TRN2", target_bir_lowering=False, debug=False,
                   num_devices=NCORES)

    hT_d = nc.dram_tensor("hT", [P, DT, T], F16, kind="ExternalInput")
    # host-prepacked, column-block-major: [q0 | q1 | g0 | g1] each [P, DT, 128]
    wqg_d = nc.dram_tensor("wqg", [2, P, DT, P], F16, kind="ExternalInput")
    wqg8_d = nc.dram_tensor("wqg8", [NT, P, DT, P], F16, kind="ExternalInput")
    hts_d = nc.dram_tensor("hts", [P, DT, TSL], F16, kind="ExternalInput")
    wkv_d = nc.dram_tensor("wkv", [2, P, DT, P], F16, kind="ExternalInput")
    wo_d = nc.dram_tensor("wo", [P, NT, 2048], F16, kind="ExternalInput")
    tblq_d = nc.dram_tensor("tblq", [2, P, T], F16, kind="ExternalInput")
    if not unit_w:
        wqk_d = nc.dram_tensor("wqk", [P, 2], F32, kind="ExternalInput")
    masks_d = nc.dram_tensor("masks", [P, max(1, mtot)], F16,
                             kind="ExternalInput")
    out_d = nc.dram_tensor("out", [TSL, D], F32, kind="ExternalOutput")

    with tile.TileContext(nc) as tc:
        with (
            tc.tile_pool(name="consts", bufs=1) as consts,
            tc.tile_pool(name="perm", bufs=1) as perm,
            tc.tile_pool(name="hw", bufs=4) as hw,
            tc.tile_pool(name="tmp", bufs=3) as tmp,
            tc.tile_pool(name="ptp", bufs=4) as ptp,
            tc.tile_pool(name="ps", bufs=1, space="PSUM") as psp,
            tc.tile_pool(name="dram", bufs=1, space="DRAM") as dram,
        ):
            # ---- resident hidden^T tiles: hTt[dt][j] = hT[:, dt, 512j:+512]
            hTt = [[consts.tile([P, 512], F16, tag=f"hT_{dt}_{j}",
                                name=f"hT_{dt}_{j}") for j in range(NJ)]
                   for dt in range(DT)]
            wq_sb = [consts.tile([P, DT, P], F16, tag=f"wq{c}", name=f"wq{c}")
                     for c in range(2)]        # q0, q1 column blocks
            hts_sb = consts.tile([P, DT, TSL], F16, tag="hts", name="hts")
            wkv_sb = [consts.tile([P, DT, P], F16, tag=f"wkv{c}",
                                  name=f"wkv{c}") for c in range(2)]  # k, v
            tb = {}
            for nm, idx in (("c", 0), ("s", 1)):
                tb[nm] = consts.tile([P, T], F16, tag=f"tb_{nm}",
                                     name=f"tb_{nm}")
            if not unit_w:
                wqk_sb = consts.tile([P, 2], F32)
                nc.sync.dma_start(wqk_sb[:], wqk_d[:])
            mask_sb = [consts.tile([P, w], F16, tag=f"mask{s}",
                                    name=f"mask{s}")
                       for s, (_, _, _, w) in enumerate(mtiles)]
            ones_sb = consts.tile([P, P], F16)
            nc.vector.memset(ones_sb[:], 1.0)
            ident_sb = consts.tile([P, P], F16)
            make_identity(nc, ident_sb[:])
            eps_sb = consts.tile([P, 1], F32)
            nc.vector.memset(eps_sb[:], EPS)
            eb_sb = consts.tile([P, 1], F32)
            nc.vector.memset(eb_sb[:], EXP_BIAS)
            zcol_sb = consts.tile([P, 1], F16)
            nc.vector.memset(zcol_sb[:], 0.0)

            # ---- persistent activations ----
            qTr = [perm.tile([P, T], F16, tag=f"qTr{h}", name=f"qTr{h}")
                   for h in range(2)]
            kTr = perm.tile([P, T], F16, tag="kTr")
            v_sb = perm.tile([P, NT, P], F16, tag="v_sb")
            rcg8 = perm.tile([P, 8, TSL], F16, tag="rcg8")  # h1 gate recips

            a2a_in = [dram.tile([NCORES * P, TSL], F16, name=f"a2a_in{h}")
                      for h in range(2)]
            a2a_in8 = [a.rearrange("(s r) t -> s r t", r=P) for a in a2a_in]
            a2a_out = [dram.tile([NCORES * P, TSL], F16, name=f"a2a_out{h}")
                       for h in range(2)]

            # ================= DMA schedule (SP FIFO order) =================
            # wkv quartiles pace the first k/v chains; all of hT streams next;
            # everything else (wq, rope tables, masks) is needed only after
            # phase A finishes (~30us) and loads behind it.
            def dma_hT(j, dts):
                for dt in dts:
                    nc.sync.dma_start(hTt[dt][j][:],
                                      hT_d[:, dt, 512 * j:512 * j + 512])

            nc.sync.dma_start(wkv_sb[0][:], wkv_d[0])
            dma_hT(0, range(DT))
            nc.sync.dma_start(wkv_sb[1][:], wkv_d[1])
            dma_hT(1, range(DT))
            for nm, idx in (("c", 0), ("s", 1)):
                nc.sync.dma_start(tb[nm][:], tblq_d[idx])
            dma_hT(2, range(DT))
            nc.sync.dma_start(wq_sb[0][:], wqg_d[0])
            dma_hT(3, range(DT))
            nc.sync.dma_start(wq_sb[1][:], wqg_d[1])
            nc.sync.dma_start(hts_sb[:], hts_d[:])
            moff = 0
            for s, (_, _, _, w) in enumerate(mtiles):
                nc.sync.dma_start(mask_sb[s][:], masks_d[:, moff:moff + w])
                moff += w
            # first wg-ring slots prefetch here; the rest stream in phase D
            wg_sb = []
            for bn in range(3):
                w_ = hw.tile([P, DT, P], F16, tag="wg", bufs=3,
                             name=f"wg_{bn}")
                nc.sync.dma_start(w_[:], wqg8_d[bn])
                wg_sb.append(w_)

            # ================= shared epilogues =================
            def rope_norm_epi(mm_ps, dest, tsl, widx):
                """rms-norm + rope from a [P,512] PSUM proj block to dest.

                qpre gets a private ring: it is the PSUM drain, and must
                never wait behind rope ops stalled on the table DMAs."""
                qpre = tmp.tile([P, 512], F16, tag="qpre", bufs=3)
                nc.vector.tensor_copy(qpre[:], mm_ps[:])
                q2 = tmp.tile([P, 512], F16, tag="tmp2", bufs=2)
                nc.scalar.activation(q2[:], mm_ps[:], AF.Square)
                if not unit_w:
                    qw = tmp.tile([P, 512], F16, tag="tmp")
                    nc.vector.tensor_scalar_mul(
                        qw[:], qpre[:], wqk_sb[:, widx:widx + 1])
                    qpre = qw
                ssq_ps = psp.tile([P, 512], F32, tag="aux", bufs=1)
                nc.tensor.matmul(ssq_ps[:], ones_sb[:], q2[:],
                                 start=True, stop=True)
                rsv = tmp.tile([P, 512], F16, tag="tmp")
                nc.scalar.activation(rsv[:], ssq_ps[:], AF.Ln,
                                     scale=1.0 / HD, bias=eps_sb[:, 0:1])
                nc.scalar.activation(rsv[:], rsv[:], AF.Exp, scale=-0.5)
                tcos = tmp.tile([P, 512], F16, tag="tmp")
                nc.vector.tensor_tensor(tcos[:], qpre[:], tb["c"][:, tsl],
                                        OP.mult)
                t2 = tmp.tile([P, 512], F16, tag="tmp")
                # sin halves pre-swapped host-side; only out is shifted
                nc.vector.tensor_tensor(t2[0:64, :], qpre[64:128, :],
                                        tb["s"][64:128, tsl], OP.mult)
                nc.vector.tensor_tensor(t2[64:128, :], qpre[0:64, :],
                                        tb["s"][0:64, tsl], OP.mult)
                nc.vector.tensor_tensor(t2[:], tcos[:], t2[:], OP.add)
                nc.vector.tensor_tensor(dest, t2[:], rsv[:], OP.mult)

            def v_epi(mm_ps, j):
                # private ring: vtmp drains PSUM and feeds PE transposes; it
                # must not queue behind rope ops stalled on table DMAs
                vtmp = tmp.tile([P, 512], F16, tag="vtmp", bufs=2)
                nc.vector.tensor_copy(vtmp[:], mm_ps[:])
                for kk in range(4):
                    tt = 4 * j + kk
                    trp = psp.tile([P, P], F16, tag="aux", bufs=1)
                    nc.tensor.transpose(
                        trp[:], vtmp[:, P * kk:P * kk + P], ident_sb[:])
                    nc.vector.tensor_copy(v_sb[:, tt, :], trp[:])

            # ================= phase A: k/v projections =================
            # j0 runs k-chain then v-chain (k needs only the first weight
            # DMA, so the PE starts ~1.5us in and the v weights hide behind
            # j0's tile stream); later chunks interleave k/v per tile.
            for j in range(NJ):
                tsl = slice(512 * j, 512 * j + 512)
                kps = psp.tile([P, 512], F32, tag="mm", bufs=3,
                               name=f"kps{j}")
                vps = psp.tile([P, 512], F32, tag="mm", bufs=3,
                               name=f"vps{j}")
                if j == 0:
                    for dt in range(DT):
                        nc.tensor.matmul(kps[:], wkv_sb[0][:, dt, :],
                                         hTt[dt][j][:],
                                         start=(dt == 0), stop=(dt == DT - 1))
                    for dt in range(DT):
                        nc.tensor.matmul(vps[:], wkv_sb[1][:, dt, :],
                                         hTt[dt][j][:],
                                         start=(dt == 0), stop=(dt == DT - 1))
                else:
                    for dt in range(DT):
                        nc.tensor.matmul(kps[:], wkv_sb[0][:, dt, :],
                                         hTt[dt][j][:],
                                         start=(dt == 0), stop=(dt == DT - 1))
                        nc.tensor.matmul(vps[:], wkv_sb[1][:, dt, :],
                                         hTt[dt][j][:],
                                         start=(dt == 0), stop=(dt == DT - 1))
                rope_norm_epi(kps, kTr[:, tsl], tsl, widx=1)
                v_epi(vps, j)

            # ================= phases per head =================
            def proj_qg(h):
                for j in range(NJ):
                    tsl = slice(512 * j, 512 * j + 512)
                    qps = psp.tile([P, 512], F32, tag="mm", bufs=3,
                                   name=f"qps{h}{j}")
                    for dt in range(DT):
                        nc.tensor.matmul(qps[:], wq_sb[h][:, dt, :],
                                         hTt[dt][j][:],
                                         start=(dt == 0), stop=(dt == DT - 1))
                    rope_norm_epi(qps, qTr[h][:, tsl], tsl, widx=0)

            def emit_attention(h, j):
                tsl = slice(512 * j, 512 * j + 512)
                valid = [i for i in range(NT) if not flags[i][j][0]]
                last = len(valid) - 1
                ot_ps = psp.tile([P, 512], F32, tag="acc", bufs=4,
                                 name=f"ot_{h}_{j}")
                # softmax denominator accumulates on the Pool engine where
                # that engine is free: all of h0 (before AllToAll#0), and
                # h1's last chunks (whose pt tiles appear only after A2A#0
                # has released Pool). h1's early chunks keep PE ones-matmuls:
                # their Pool ops would queue behind the 28us collective and
                # back-pressure the pt ring into a PE stall.
                pool_rs = (h == 0)
                if pool_rs:
                    rsacc = tmp.tile([P, 512], F16, tag="rsacc", bufs=2,
                                     name=f"rsacc_{h}_{j}")
                else:
                    rs_ps = psp.tile([P, 512], F32, tag="acc", bufs=4,
                                     name=f"rs_{h}_{j}")
                DEPTH = 3  # st matmuls emitted ahead of their PV consumers
                pts = {}

                def win(idx):
                    # live column window; first tile full-width (PSUM init)
                    if idx == 0:
                        return slice(0, 512)
                    _, _, _, c0, c1 = flags[valid[idx]][j]
                    return slice(c0, c1)

                def front(idx):
                    i = valid[idx]
                    w = win(idx)
                    st_ps = psp.tile([P, 512], F32, tag="mm", bufs=3,
                                     name=f"st_{h}_{j}_{i}")
                    nc.tensor.matmul(st_ps[:, w], kTr[:, P * i:P * i + P],
                                     qTr[h][:, 512 * j + w.start:
                                            512 * j + w.stop],
                                     start=True, stop=True)
                    pt = ptp.tile([P, 512], F16, tag="pt",
                                  name=f"pt_{h}_{j}_{i}")
                    nc.scalar.activation(pt[:, w], st_ps[:, w], AF.Exp,
                                         bias=eb_sb[:, 0:1])
                    if (i, j) in mslot:
                        ms, mb, _ = mslot[(i, j)]
                        nc.vector.tensor_tensor(
                            pt[:, w], pt[:, w],
                            mask_sb[ms][:, w.start - mb:w.stop - mb],
                            OP.mult)
                    pts[idx] = pt

                def back(idx):
                    i = valid[idx]
                    w = win(idx)
                    pt = pts.pop(idx)
                    nc.tensor.matmul(ot_ps[:, w], v_sb[:, i, :], pt[:, w],
                                     start=(idx == 0), stop=(idx == last))
                    if pool_rs:
                        if idx == 0:
                            nc.gpsimd.tensor_copy(rsacc[:], pt[:])
                        else:
                            nc.gpsimd.tensor_tensor(rsacc[:, w], rsacc[:, w],
                                                    pt[:, w], OP.add)
                    else:
                        nc.tensor.matmul(rs_ps[:, w], ones_sb[:], pt[:, w],
                                         start=(idx == 0), stop=(idx == last))

                for idx in range(len(valid)):
                    front(idx)
                    if idx >= DEPTH - 1:
                        back(idx - DEPTH + 1)
                for idx in range(max(0, len(valid) - DEPTH + 1), len(valid)):
                    back(idx)
                if pool_rs:
                    rsall = tmp.tile([P, 512], F32, tag="rsall", bufs=1,
                                     name=f"rsall_{h}_{j}")
                    nc.gpsimd.partition_all_reduce(
                        rsall[:], rsacc[:], channels=P,
                        reduce_op=bass_isa.ReduceOp.add)
                    rs_ap = rsall[:]
                else:
                    rs_ap = rs_ps[:]

                # atg = ot / rowsum (gating applied at the destination);
                # per 256-col half so each shard's staging DMA launches as
                # soon as its half of the epilogue is done
                sgm = tmp.tile([P, 512], F32, tag="tmp", name=f"sgm_{h}_{j}")
                atg = tmp.tile([P, 512], F16, tag="tmp2", bufs=2,
                               name=f"atg_{h}_{j}")
                for half in range(2):
                    hs = slice(256 * half, 256 * half + 256)
                    nc.vector.reciprocal_approx_fast(sgm[:, hs], rs_ap[:, hs])
                    nc.vector.tensor_tensor(atg[:, hs], ot_ps[:, hs],
                                            sgm[:, hs], OP.mult)
                    nc.sync.dma_start(
                        a2a_in8[h][2 * j + half, :, :], atg[:, hs])
                return atg

            def a2a(h):
                if use_collective:
                    nc.gpsimd.collective_compute(
                        "AllToAll", OP.bypass,
                        replica_groups=[list(range(NCORES))],
                        ins=[a2a_in[h][:].opt()], outs=[a2a_out[h][:].opt()])
                else:
                    nc.sync.dma_start(a2a_out[h][:], a2a_in[h][:])

            proj_qg(0)
            for j in range(NJ):
                emit_attention(0, j)
            a2a(0)
            # wo prefetch, first half only: in the SP queue after h0's
            # staging and BEFORE h1's (the second half is emitted in phase D
            # so h1's staging DMAs are never queued behind it)
            wo_sb = []
            for ht in range(8):
                w_ = hw.tile([P, 2048], F16, tag="wo", bufs=8, name=f"wo_{ht}")
                nc.sync.dma_start(w_[:], wo_d[:, ht, :])
                wo_sb.append(w_)
            proj_qg(1)
            last_atg = None
            for j in range(NJ):
                last_atg = emit_attention(1, j)
            a2a(1)

            # ================= o-proj =================
            # 8 PSUM banks accumulate [m 0/1] x [Dc 0..3] over all 16 ht
            # blocks; part 1 (ht 0-7, from a2a_out[0]) runs while A2A#1 is in
            # flight; part 2 runs Dc-major so assembly overlaps the tail.
            ops_tags = ["mm", "mm", "mm", "aux", "acc", "acc", "acc", "acc"]
            ops_bufs = {"mm": 3, "aux": 1, "acc": 4}
            ops = []
            for m in range(2):
                for Dc in range(NJ):
                    tg = ops_tags[m * NJ + Dc]
                    ops.append(psp.tile([P, 512], F32, tag=tg,
                                        bufs=ops_bufs[tg],
                                        name=f"ops{m}_{Dc}"))
            at0 = perm.tile([P, 8, TSL], F16, tag="ATall0", name="ATall0")
            nc.sync.dma_start(
                at0[:], a2a_out[0].rearrange("(i r) t -> r i t", r=P))
            # order the gating (and transitively all o-proj matmuls) behind
            # h1's attention with a REAL data dep: a 1.0 derived from h1's
            # last atg tile scales at0's first block in place
            one1 = tmp.tile([P, 1], F32, tag="rcg0", bufs=2, name="one1")
            nc.vector.tensor_scalar(one1[:], last_atg[:, 0:1], 0.0, 1.0,
                                    OP.mult, OP.add)
            nc.vector.tensor_scalar_mul(at0[:, 0, :], at0[:, 0, :], one1[:])

            # ---- destination-side gating: per received head block ht,
            # project THIS core's 256 hidden rows through that head's gate
            # columns (16 matmuls of [P,256]), then scale the block by
            # sigmoid = 1/(1+e^-gate) in place. All projections run during
            # the A2A windows; h0 blocks gate as soon as rcpg is ready,
            # h1 blocks gate when at1 lands.
            rcpgs = []
            for bn in range(NT):
                if bn >= 3:
                    w_ = hw.tile([P, DT, P], F16, tag="wg", bufs=3,
                                 name=f"wg_{bn}")
                    nc.sync.dma_start(w_[:], wqg8_d[bn])
                    wg_sb.append(w_)
                gate_ps = psp.tile([P, TSL], F32, tag="mm", bufs=3,
                                   name=f"gps_{bn}")
                for dt in range(DT):
                    nc.tensor.matmul(gate_ps[:], wg_sb[bn][:, dt, :],
                                     hts_sb[:, dt, :],
                                     start=(dt == 0), stop=(dt == DT - 1))
                eg = tmp.tile([P, TSL], F16, tag="tmp", name=f"eg_{bn}")
                nc.scalar.activation(eg[:], gate_ps[:], AF.Exp, scale=-1.0)
                gp1 = tmp.tile([P, TSL], F32, tag="tmp", name=f"gp1_{bn}")
                nc.vector.tensor_scalar_add(gp1[:], eg[:], 1.0)
                if bn < 8:
                    rcpg = tmp.tile([P, TSL], F32, tag="rcg0", bufs=2,
                                    name=f"rcpg_{bn}")
                    nc.vector.reciprocal_approx_fast(rcpg[:], gp1[:])
                    nc.vector.tensor_tensor(at0[:, bn, :], at0[:, bn, :],
                                            rcpg[:], OP.mult)
                else:
                    rcpg = tmp.tile([P, TSL], F32, tag="rcg0", bufs=2,
                                    name=f"rcpg_{bn}")
                    nc.vector.reciprocal_approx_fast(rcpg[:], gp1[:])
                    nc.vector.tensor_copy(rcg8[:, bn - 8, :], rcpg[:])
            # second wo half loads here: after h1's staging in the SP queue,
            # paced by o-proj part 1 freeing ring slots
            for ht in range(8, NT):
                w_ = hw.tile([P, 2048], F16, tag="wo", bufs=8, name=f"wo_{ht}")
                nc.sync.dma_start(w_[:], wo_d[:, ht, :])
                wo_sb.append(w_)
            at1 = perm.tile([P, 8, TSL], F16, tag="ATall1", name="ATall1")
            nc.sync.dma_start(
                at1[:], a2a_out[1].rearrange("(i r) t -> r i t", r=P))
            for bn in range(8, NT):
                nc.vector.tensor_tensor(at1[:, bn - 8, :], at1[:, bn - 8, :],
                                        rcg8[:, bn - 8, :], OP.mult)
            ATall = [at0, at1]

            def oproj_mm(ht, Dc, m):
                nc.tensor.matmul(
                    ops[m * NJ + Dc][:],
                    ATall[ht // 8][:, ht % 8, P * m:P * m + P],
                    wo_sb[ht][:, 512 * Dc:512 * Dc + 512],
                    start=(ht == 0), stop=(ht == NT - 1))

            for ht in range(8):
                for Dc in range(NJ):
                    for m in range(2):
                        oproj_mm(ht, Dc, m)
            # part 2: Dc-major so each closed [m, Dc] block drains and
            # streams out while later Dc blocks still accumulate
            for Dc in range(NJ):
                for m in range(2):
                    for ht in range(8, NT):
                        oproj_mm(ht, Dc, m)
                for m in range(2):
                    o_sb = hw.tile([P, 512], F32, tag="osb", bufs=2,
                                   name=f"o_{m}_{Dc}")
                    nc.vector.tensor_copy(o_sb[:], ops[m * NJ + Dc][:])
                    nc.sync.dma_start(
                        out_d[P * m:P * m + P, 512 * Dc:512 * Dc + 512],
                        o_sb[:])

    nc.compile()
    _dedupe_act_table_loads(nc)
    return nc


def _dedupe_act_table_loads(nc):
    """Bacc assigns Exp->exp_and_others and Ln->natural_log, inserting a
    ~2.7us table load at every Exp<->Ln alternation. All activation funcs
    this kernel uses (Exp, Ln, Square) live in the natural_log_exp_and_others
    set, so keep one load of that set and drop the rest."""
    from concourse.hw_specs import get_activation_tables
    tabs = list(get_activation_tables(nc.m.arch).items())
    nl_exp = next(i for i, (nm, funcs) in enumerate(tabs)
                  if nm == "natural_log_exp_and_others")
    used = {ins.func for bb in nc.main_func.blocks for ins in bb.instructions
            if isinstance(ins, mybir.InstActivation)}
    assert used <= tabs[nl_exp][1], f"funcs {used} not all in natural_log_exp"
    first = True
    for bb in nc.main_func.blocks:
        keep = []
        for ins in bb.instructions:
            if isinstance(ins, mybir.InstLoadActFuncSet):
                assert ins.sync_info is None or (
                    not ins.sync_info.on_wait and not ins.sync_info.on_update)
                if first:
                    ins.act_func_set_id = nl_exp
                    keep.append(ins)
                    first = False
                continue
            keep.append(ins)
        bb.instructions[:] = keep


def _host_prep(hidden_BTD, cos_BTK, sin_BTK, segment_ids_BT, position_ids_BT,
               wq, wk, wv, wo, q_norm_w, k_norm_w):
    hidden = np.ascontiguousarray(np.asarray(hidden_BTD, dtype=np.float32)[0])
    cos = np.asarray(cos_BTK, dtype=np.float32)[0]
    sin = np.asarray(sin_BTK, dtype=np.float32)[0]
    seg = np.asarray(segment_ids_BT)[0]
    pos = np.asarray(position_ids_BT)[0]
    wq = np.asarray(wq, dtype=np.float32)
    wk = np.asarray(wk, dtype=np.float32)
    wv = np.asarray(wv, dtype=np.float32)
    wo = np.asarray(wo, dtype=np.float32)
    q_norm_w = np.asarray(q_norm_w, dtype=np.float32)
    k_norm_w = np.asarray(k_norm_w, dtype=np.float32)

    assert np.array_equal(pos, np.arange(T, dtype=pos.dtype)), \
        "kernel assumes position_ids == arange"
    assert np.all(np.diff(seg) >= 0), "kernel assumes sorted segment ids"

    # hT[p, dt, t] = hidden[t, 128*dt + p]
    hT = np.ascontiguousarray(
        hidden.T.reshape(DT, P, T).transpose(1, 0, 2).astype(np.float16))
    sqrtS = np.float32(np.sqrt(SCALE))
    signv = np.where(np.arange(HD) < HD // 2, -1.0, 1.0).astype(np.float32)
    shuf = (np.arange(HD) + HD // 2) % HD

    cosw = (cos.T * sqrtS).astype(np.float32)
    sinw = (sin.T * signv[:, None] * sqrtS).astype(np.float32)
    sinswap = sinw[shuf]  # halves swapped: see rope ops in _build_program
    tblq = np.ascontiguousarray(np.stack([cosw, sinswap]).astype(np.float16))
    unit_w = bool(np.all(q_norm_w == 1.0) and np.all(k_norm_w == 1.0))
    wqk = np.ascontiguousarray(np.stack([q_norm_w, k_norm_w], axis=1))

    # prepack wo: partition-major, block order = o-proj ht order
    # (all h0 head-blocks, then all h1)
    perm = [2 * i + h for h in range(2) for i in range(NCORES)]
    wo_p = wo.reshape(NT, P, 2048)[perm].transpose(1, 0, 2)
    wo_p = np.ascontiguousarray(wo_p.astype(np.float16))

    seg_end = np.searchsorted(seg, seg, side="right").astype(np.int64)
    flags = _tile_flags(seg_end)
    mtiles = _masked_tiles(flags)
    # packed mask windows: keep iff 128i+p <= 512j+t < seg_end[128i+p]
    mtot = sum(w for _, _, _, w in mtiles)
    masks = np.zeros((P, max(1, mtot)), dtype=np.float16)
    moff = 0
    for (i, j, b, w) in mtiles:
        sglob = P * i + np.arange(P)
        tg = 512 * j + b + np.arange(w)
        keep = (sglob[:, None] <= tg[None, :]) & \
               (tg[None, :] < seg_end[sglob][:, None])
        masks[:, moff:moff + w] = keep.astype(np.float16)
        moff += w

    # gate columns for ALL heads, in o-proj block order (shared by cores)
    gperm = [2 * i + hh for hh in range(2) for i in range(NCORES)]
    wqg8 = np.stack([
        wq[:, ht * 256 + 128: ht * 256 + 256].reshape(DT, P, P)
        .transpose(1, 0, 2) for ht in gperm])
    wqg8 = np.ascontiguousarray(wqg8.astype(np.float16))

    in_maps = []
    for c in range(NCORES):
        h0, h1 = 2 * c, 2 * c + 1
        g = c // 2
        cols = [wq[:, h0 * 256: h0 * 256 + 128],
                wq[:, h1 * 256: h1 * 256 + 128]]
        wqg_p = np.stack([cb.reshape(DT, P, P).transpose(1, 0, 2)
                          for cb in cols])
        wqg_p = np.ascontiguousarray(wqg_p.astype(np.float16))
        kvcols = [wk[:, g * 128:(g + 1) * 128], wv[:, g * 128:(g + 1) * 128]]
        wkv_p = np.stack([cb.reshape(DT, P, P).transpose(1, 0, 2)
                          for cb in kvcols])
        wkv_p = np.ascontiguousarray(wkv_p.astype(np.float16))
        # this core's 256 hidden^T columns, for destination-side gating
        hts = np.ascontiguousarray(hT[:, :, TSL * c:TSL * (c + 1)])
        m = {
            "hT": hT, "wqg": wqg_p, "wkv": wkv_p, "wo": wo_p,
            "tblq": tblq, "masks": masks, "wqg8": wqg8, "hts": hts,
        }
        if not unit_w:
            m["wqk"] = wqk
        in_maps.append(m)
    return in_maps, seg_end, unit_w


def kernel(**inputs) -> np.ndarray:
    in_maps, seg_end, unit_w = _host_prep(**inputs)
    key = (_tile_flags(seg_end), unit_w)
    if key not in _program_cache:
        _program_cache[key] = _build_program(key)
    nc = _program_cache[key]
    res = run_bass_kernel_spmd(nc, in_maps, list(range(NCORES)))
    out = np.concatenate([res.results[c]["out"] for c in range(NCORES)], axis=0)
    return out[None].astype(np.float32)
